# revision 2
# baseline (speedup 1.0000x reference)
"""Trainium2 Bass kernel for nn_AVDFullLinearMix.

Math (folded form, all terms single matmuls over raw inputs):
  x_d_out = x_d + W_ttrans @ x_d^T(spatial) + (W_tdelta @ x_a) * eye3
            + eps_expand(W_vd @ x_v)
  x_a_out = x_a + W_ct @ trace(x_d),        W_ct = W_ttrace @ (I + W_ttrans)
  x_v_out = x_v + W_cd @ eps_contract(x_d), W_cd = W_dv @ (I - W_ttrans)
(the TensDelta term never reaches the eps-contract since eps[i,i,k] = 0,
 and trace/eps-contract commute with the TensTrans spatial transpose up to
 identity/negation, so both weight chains fold on the host.)

Sharding: data-parallel over 8 cores (2048 tokens each); the five small
weight matrices are replicated. Device layout is channel-major
[256, chunk, spatial, token] so every DMA is contiguous per partition and
every matmul consumes [128 x T] tiles directly. Matmuls run as float32r
(TF32-like fp22 multiply, fp32 accumulate) at full PE rate.
"""

import numpy as np

import concourse.bass as bass  # noqa: F401  (registers handle types)
import concourse.mybir as mybir
import concourse.tile as tile
from concourse import bacc
from concourse.bass_utils import run_bass_kernel_spmd

NCORES = 8
P = 128          # partitions
C = 256          # channels
B, N = 16, 1024
T = (B * N) // NCORES   # tokens per core = 2048
TCH = 256               # tokens per chunk
NCHUNK = T // TCH       # 8

F32 = mybir.dt.float32
R32 = mybir.dt.float32r

# flat spatial index s = i*3 + j
SPERM = [0, 3, 6, 1, 4, 7, 2, 5, 8]    # s -> transposed flat (j*3+i)
DIAG = (0, 4, 8)
# off-diag s=(k,j): x_d_out[...,k,j] += eps[i*,j,k] * (W_vd @ x_v[..,i*])
# VD_MAP: s -> (i*, weight slot)  where slot 2 = +W_vd^T, 3 = -W_vd^T
VD_MAP = {1: (2, 3), 2: (1, 2), 3: (2, 2), 5: (0, 3), 6: (1, 3), 7: (0, 2)}
# eps-contract y[:,k] = x_d[:,s1] - x_d[:,s2]
Y_PAIRS = [(5, 7), (6, 2), (1, 3)]

_CACHE: dict = {}


def _build():
    nc = bacc.Bacc(None, target_bir_lowering=False)
    xa = nc.dram_tensor("xa", [C, NCHUNK, TCH], F32, kind="ExternalInput")
    xv = nc.dram_tensor("xv", [C, NCHUNK, 3, TCH], F32, kind="ExternalInput")
    xd = nc.dram_tensor("xd", [C, NCHUNK, 9, TCH], F32, kind="ExternalInput")
    wts = nc.dram_tensor("wts", [6, C, C], F32, kind="ExternalInput")
    xa_o = nc.dram_tensor("xa_o", [C, NCHUNK, TCH], F32, kind="ExternalOutput")
    xv_o = nc.dram_tensor("xv_o", [C, NCHUNK, 3, TCH], F32, kind="ExternalOutput")
    xd_o = nc.dram_tensor("xd_o", [C, NCHUNK, 9, TCH], F32, kind="ExternalOutput")

    # channel-split dram views: c = h*128 + p
    xa_r = xa.rearrange("(h p) n t -> p h n t", p=P)
    xv_r = xv.rearrange("(h p) n k t -> p h n k t", p=P)
    xd_r = xd.rearrange("(h p) n s t -> p h n s t", p=P)
    xa_or = xa_o.rearrange("(h p) n t -> p h n t", p=P)
    xv_or = xv_o.rearrange("(h p) n k t -> p h n k t", p=P)
    xd_or = xd_o.rearrange("(h p) n s t -> p h n s t", p=P)
    wts_r = wts.rearrange("w (kh p) o -> p w kh o", p=P)

    with tile.TileContext(nc) as tc:
        with (
            tc.tile_pool(name="wpool", bufs=1) as wpool,
            tc.tile_pool(name="data", bufs=2) as data,
            tc.tile_pool(name="outp", bufs=2) as outp,
            tc.tile_pool(name="tmp", bufs=2) as tmp,
            tc.tile_pool(name="psum", bufs=8, space="PSUM") as psum,
        ):
            w_sb = wpool.tile([P, 6, 2, C], F32)
            nc.sync.dma_start(w_sb[:].bitcast(R32), wts_r.bitcast(R32))

            def lhsT(w, kh, mh):
                return w_sb[:, w, kh, mh * P:(mh + 1) * P].bitcast(R32)

            for ck in range(NCHUNK):
                xd_sb = data.tile([P, 2, 9, TCH], F32, tag="xd")
                xv_sb = data.tile([P, 2, 3, TCH], F32, tag="xv")
                xa_sb = data.tile([P, 2, TCH], F32, tag="xa")
                nc.sync.dma_start(xd_sb[:].bitcast(R32), xd_r[:, :, ck].bitcast(R32))
                nc.sync.dma_start(xv_sb[:].bitcast(R32), xv_r[:, :, ck].bitcast(R32))
                nc.sync.dma_start(xa_sb[:].bitcast(R32), xa_r[:, :, ck].bitcast(R32))
                xd_ob = outp.tile([P, 2, 9, TCH], F32, tag="xd_o")
                xv_ob = outp.tile([P, 2, 3, TCH], F32, tag="xv_o")
                xa_ob = outp.tile([P, 2, TCH], F32, tag="xa_o")

                # eps-contract and trace of raw x_d; written as fp32r so the
                # consuming matmuls pass the fp32r-producer check
                y_sb = tmp.tile([P, 2, 3, TCH], F32, tag="y")
                tr_sb = tmp.tile([P, 2, TCH], F32, tag="tr")
                for k, (s1, s2) in enumerate(Y_PAIRS):
                    nc.vector.tensor_sub(
                        y_sb[:, :, k, :].bitcast(R32),
                        xd_sb[:, :, s1, :], xd_sb[:, :, s2, :],
                    )
                nc.vector.tensor_add(
                    tr_sb[:].bitcast(R32), xd_sb[:, :, 0, :], xd_sb[:, :, 4, :]
                )
                nc.vector.tensor_add(
                    tr_sb[:].bitcast(R32), tr_sb[:], xd_sb[:, :, 8, :]
                )

                # x_d path: per spatial s and output-channel half mh,
                # psum = W_ttrans @ xd^T + (delta | eps-expand vd)
                for s in range(9):
                    for mh in range(2):
                        ps = psum.tile([P, TCH], F32, tag="ps")
                        for kh in range(2):
                            nc.tensor.matmul(
                                ps[:], lhsT(0, kh, mh),
                                xd_sb[:, kh, SPERM[s], :].bitcast(R32),
                                start=(kh == 0), stop=False,
                            )
                        if s in DIAG:
                            for kh in range(2):
                                nc.tensor.matmul(
                                    ps[:], lhsT(1, kh, mh),
                                    xa_sb[:, kh, :].bitcast(R32),
                                    start=False, stop=(kh == 1),
                                )
                        else:
                            istar, wslot = VD_MAP[s]
                            for kh in range(2):
                                nc.tensor.matmul(
                                    ps[:], lhsT(wslot, kh, mh),
                                    xv_sb[:, kh, istar, :].bitcast(R32),
                                    start=False, stop=(kh == 1),
                                )
                        nc.vector.tensor_add(
                            xd_ob[:, mh, s, :], xd_sb[:, mh, s, :], ps[:]
                        )

                # x_v path: xv += W_cd @ y
                for k in range(3):
                    for mh in range(2):
                        ps = psum.tile([P, TCH], F32, tag="ps")
                        for kh in range(2):
                            nc.tensor.matmul(
                                ps[:], lhsT(4, kh, mh),
                                y_sb[:, kh, k, :].bitcast(R32),
                                start=(kh == 0), stop=(kh == 1),
                            )
                        nc.vector.tensor_add(
                            xv_ob[:, mh, k, :], xv_sb[:, mh, k, :], ps[:]
                        )

                # x_a path: xa += W_ct @ tr
                for mh in range(2):
                    ps = psum.tile([P, TCH], F32, tag="ps")
                    for kh in range(2):
                        nc.tensor.matmul(
                            ps[:], lhsT(5, kh, mh),
                            tr_sb[:, kh, :].bitcast(R32),
                            start=(kh == 0), stop=(kh == 1),
                        )
                    nc.vector.tensor_add(xa_ob[:, mh, :], xa_sb[:, mh, :], ps[:])

                nc.sync.dma_start(xd_or[:, :, ck], xd_ob[:])
                nc.sync.dma_start(xv_or[:, :, ck], xv_ob[:])
                nc.sync.dma_start(xa_or[:, :, ck], xa_ob[:])
    nc.compile()
    return nc


def _get_nc():
    if "nc" not in _CACHE:
        _CACHE["nc"] = _build()
    return _CACHE["nc"]


def kernel(x_a, x_v, x_d, W_ttrans, W_ttrace, W_tdelta, W_vd, W_dv, **_ignored):
    x_a = np.ascontiguousarray(np.asarray(x_a, dtype=np.float32))
    x_v = np.ascontiguousarray(np.asarray(x_v, dtype=np.float32))
    x_d = np.ascontiguousarray(np.asarray(x_d, dtype=np.float32))
    W_ttrans = np.asarray(W_ttrans, dtype=np.float32)
    W_ttrace = np.asarray(W_ttrace, dtype=np.float32)
    W_tdelta = np.asarray(W_tdelta, dtype=np.float32)
    W_vd = np.asarray(W_vd, dtype=np.float32)
    W_dv = np.asarray(W_dv, dtype=np.float32)

    eye = np.eye(C, dtype=np.float32)
    W_ct = (W_ttrace @ (eye + W_ttrans)).astype(np.float32)
    W_cd = (W_dv @ (eye - W_ttrans)).astype(np.float32)
    wts = np.ascontiguousarray(
        np.stack([
            W_ttrans.T, W_tdelta.T, W_vd.T, -W_vd.T, W_cd.T, W_ct.T,
        ]).astype(np.float32)
    )

    # host reshard to channel-major per-core chunked layout
    xd_dev = np.ascontiguousarray(
        x_d.reshape(NCORES, NCHUNK, TCH, C, 9).transpose(0, 3, 1, 4, 2)
    )
    xv_dev = np.ascontiguousarray(
        x_v.reshape(NCORES, NCHUNK, TCH, C, 3).transpose(0, 3, 1, 4, 2)
    )
    xa_dev = np.ascontiguousarray(
        x_a.reshape(NCORES, NCHUNK, TCH, C).transpose(0, 3, 1, 2)
    )

    in_maps = [
        {"xa": xa_dev[c], "xv": xv_dev[c], "xd": xd_dev[c], "wts": wts}
        for c in range(NCORES)
    ]

    nc = _get_nc()
    res = run_bass_kernel_spmd(nc, in_maps, core_ids=list(range(NCORES)))

    xa_o = np.stack([res.results[c]["xa_o"] for c in range(NCORES)])
    xv_o = np.stack([res.results[c]["xv_o"] for c in range(NCORES)])
    xd_o = np.stack([res.results[c]["xd_o"] for c in range(NCORES)])

    x_a_out = xa_o.transpose(0, 2, 3, 1).reshape(B, N, C)
    x_v_out = xv_o.transpose(0, 2, 4, 1, 3).reshape(B, N, C, 3)
    x_d_out = xd_o.transpose(0, 2, 4, 1, 3).reshape(B, N, C, 3, 3)
    return (
        np.ascontiguousarray(x_a_out),
        np.ascontiguousarray(x_v_out),
        np.ascontiguousarray(x_d_out),
    )
